# revision 25
# baseline (speedup 1.0000x reference)
"""Trainium2 Bass kernel for nn_MI_35115652612725 (mutual-information loss).

Math (see reference): per h-slice,
  xs = softmax(x_seen[.,h]/T, -1)  (h, N, C1),  xu = softmax(x_unseen/T, -1)^T
  p_joint = xu @ xs / N;  p_seen/p_unseen are its column/row marginals.
  out = mean_h[ -sum p_joint*(log p_joint - log p_seen) + sum p_unseen*log p_unseen ]

Sharding: 8 cores = (h=4) x (v=2).  Core 2h+v processes the contiguous slabs
x_seen[v,h] (2048x1024) and x_unseen[v,h] (2048x2048); the N=v*B contraction
axis splits exactly along v.  Inputs are cast to fp16 on the host (the exp
argument 20*x loses <0.06 absolute, negligible through the softmax), halving
input HBM traffic.  Both matmul operands are row-normalized softmax numerators quantized to
fp8e4 (eu8 = eu/su, xsw8 = es/ss in [0,1]), run as DoubleRow fp8 matmuls
(256-deep contraction per instruction, 2x bf16 throughput), so the PSUM
partials equal N*p_joint.  p_unseen partials come from a ones-stationary
DoubleRow matmul (column sums of eu8).  Four pipelined ReduceScatters over core
pairs {2h,2h+1} sum the v-partials in bf16 and split the k axis for the
entropy phase.  The entropy phase works on the N-scaled values: clamps move
to N*EPS, the log N offsets cancel exactly in a1-b1, p_unseen's term
subtracts ln N explicitly, and the host divides the summed scalars by H*N.

A fixed -90 shift replaces the per-row max in softmax: any per-row constant
cancels exactly in all outputs here, and with |20*x| < 115 neither exp nor
the f32 sums can overflow (underflow only kills terms < e^-60 of the row
max, far below the fp8 quantization noise anyway).
"""

import math

import numpy as np

import concourse.bass as bass
import concourse.bacc as bacc
import concourse.mybir as mybir
from concourse import tile
from concourse.tile import add_dep_helper
from concourse.bass_utils import run_bass_kernel_spmd

F32 = mybir.dt.float32
F16 = mybir.dt.float16
BF16 = mybir.dt.bfloat16
FP8 = mybir.dt.float8e4
AF = mybir.ActivationFunctionType
ALU = mybir.AluOpType
AX = mybir.AxisListType
DR = mybir.MatmulPerfMode.DoubleRow

V, H, B, C1, C2 = 2, 4, 2048, 1024, 2048
N = V * B
P = 128
NST = B // (2 * P)     # 8 super-tiles of the contraction axis (256 rows each)
KB = C1 // P           # 8 stationary blocks (k on psum partitions)
CH = C2 // 512         # 4 moving chunks (c on psum free axis)
SCALE = 20.0           # 1/TEMP
SHIFT = -90.0
EPS = 1e-7
NEPS = float(EPS * N)  # clamp for the N-scaled p_joint / p_unseen values
LN_N = float(math.log(N))

NG = 4                 # pipelined collective groups (2 kb blocks each)
KBG = KB // NG         # kb blocks per group (2)
SUBPJ = P * C2         # per-rank pjT piece: 128*2048 elems
PUNW = C2 // NG // 2   # per-rank p_unseen piece: 256
SUBCH = SUBPJ + PUNW

_NC = None
LAST_RESULTS = None


def _build_nc():
    nc = bacc.Bacc(None, num_devices=8)
    # Register the exp shift as a preamble const AP (memset + barrier before
    # any tile instruction) so the Exp activations don't pick up an extra
    # sync-wait on a bias-producing instruction — the ACT instruction
    # encoding only has room for one wait here.
    shift_t = nc.alloc_sbuf_tensor(f"const-float32-{SHIFT}", [128, 1], F32)
    nc.gpsimd.memset(shift_t.ap(), SHIFT)
    nc.const_aps.aps[(F32, SHIFT)] = shift_t.ap()
    nc.all_engine_barrier()

    xs_d = nc.dram_tensor("xs", [B, C1], F16, kind="ExternalInput")
    xu_d = nc.dram_tensor("xu", [B, C2], F16, kind="ExternalInput")
    out_d = nc.dram_tensor("parts", [1, 2], F32, kind="ExternalOutput")

    with tile.TileContext(nc) as tc:
        with (
            tc.tile_pool(name="dram", bufs=1, space="DRAM") as dram,
            tc.tile_pool(name="xu_raw", bufs=2) as pool_xu,
            tc.tile_pool(name="xs_raw", bufs=2) as pool_xs,
            tc.tile_pool(name="eb", bufs=2) as pool_eb,
            tc.tile_pool(name="sb", bufs=2) as pool_sb,
            tc.tile_pool(name="eu8", bufs=NST) as pool_eu8,
            tc.tile_pool(name="xsw8", bufs=NST) as pool_xsw8,
            tc.tile_pool(name="stat", bufs=8 * NST) as stat,
            tc.tile_pool(name="psum", bufs=6, space="PSUM") as psum,
            tc.tile_pool(name="psum_fin", bufs=1, space="PSUM") as psum_fin,
            tc.tile_pool(name="psum_pu", bufs=1, space="PSUM") as psum_pu,
            tc.tile_pool(name="pjt", bufs=2) as pool_pjt,
            tc.tile_pool(name="pcl", bufs=2) as pool_pcl,
            tc.tile_pool(name="lp", bufs=1) as pool_lp,
            tc.tile_pool(name="evict", bufs=4) as pool_ev,
            tc.tile_pool(name="evict_pu", bufs=2) as pool_evpu,
            tc.tile_pool(name="pu3", bufs=2) as pool_pu3,
            tc.tile_pool(name="acc", bufs=1) as acc,
        ):
            rs_in = [
                dram.tile([2 * SUBCH], BF16, name=f"rs_in{g}") for g in range(NG)
            ]
            rs_out = [dram.tile([SUBCH], BF16, name=f"rs_out{g}") for g in range(NG)]

            # super-tile s, sub-row i, partition p covers input row
            # n = 256*s + 128*i + p — the DoubleRow contraction pairing.
            xu_t = xu_d[:].rearrange("(s i p) c -> s p i c", i=2, p=P)
            xs_t = xs_d[:].rearrange("(s i p) c -> s p i c", i=2, p=P)

            # fp8 ones stationary for the p_unseen column-sum matmul.  The
            # dual-fp8 ldweights ISA check rejects narrow stationaries, so
            # use the same [128, 2, 128] shape as the pjT stationaries — all
            # 128 output rows hold the identical column sum; evict row 0.
            ones8 = acc.tile([P, 2, P], FP8)
            nc.vector.memset(ones8[:], 1.0)

            # ---------------- phase 1: exp + normalize to fp8 ----------------
            eu8_tiles, xsw8_tiles = [], []
            last_norm = {}
            for s in range(NST):
                xu_raw = pool_xu.tile([P, 2, C2], F16)
                nc.sync.dma_start(xu_raw[:], xu_t[s])
                xs_raw = pool_xs.tile([P, 2, C1], F16)
                nc.sync.dma_start(xs_raw[:], xs_t[s])

                eu8 = pool_eu8.tile([P, 2, C2], FP8, tag="eu8", name=f"eu8_{s}")
                xsw8 = pool_xsw8.tile([P, 2, C1], FP8, tag="xsw8", name=f"xsw8_{s}")
                for i in range(2):
                    eb = pool_eb.tile([P, C2], BF16)
                    su = stat.tile([P, 1], F32, tag="stat", name=f"su{s}_{i}")
                    nc.scalar.activation(
                        eb[:], xu_raw[:, i, :], AF.Exp,
                        bias=SHIFT, scale=SCALE, accum_out=su[:],
                    )
                    wu = stat.tile([P, 1], F32, tag="stat", name=f"wu{s}_{i}")
                    nc.vector.reciprocal(wu[:], su[:])
                    nc.vector.tensor_scalar_mul(eu8[:, i, :], eb[:], wu[:])

                    sb = pool_sb.tile([P, C1], BF16)
                    ss = stat.tile([P, 1], F32, tag="stat", name=f"ss{s}_{i}")
                    nc.scalar.activation(
                        sb[:], xs_raw[:, i, :], AF.Exp,
                        bias=SHIFT, scale=SCALE, accum_out=ss[:],
                    )
                    ws = stat.tile([P, 1], F32, tag="stat", name=f"ws{s}_{i}")
                    nc.vector.reciprocal(ws[:], ss[:])
                    ts_i = nc.vector.tensor_scalar_mul(xsw8[:, i, :], sb[:], ws[:])
                    if s == NST - 2:
                        last_norm["i"] = ts_i

                eu8_tiles.append(eu8)
                xsw8_tiles.append(xsw8)

            # CC-stream warm-keeper: the first collective after the stream
            # idles pays an ~11.5us start delay.  Fire a 4-byte pair
            # AllReduce gated on a late phase-1 normalize so the stream is
            # awake when group 0's ReduceScatter triggers.
            warm_in = dram.tile([1, 1], F32, name="warm_in")
            warm_out = dram.tile([1, 1], F32, name="warm_out")
            wz = acc.tile([1, 1], F32)
            nc.vector.memset(wz[:], 0.0)
            wdma = nc.sync.dma_start(warm_in[:], wz[:])
            add_dep_helper(wdma.ins, last_norm["i"].ins, sync=True,
                           reason="warm-keeper fires late in phase 1")
            nc.gpsimd.collective_compute(
                "AllReduce",
                ALU.add,
                replica_groups=[[0, 1], [2, 3], [4, 5], [6, 7]],
                ins=[warm_in.opt()],
                outs=[warm_out.opt()],
            )

            # ---------------- phase 2 + overlapped collectives ----------------
            # group g = kb block g: pjT rows [g*128, (g+1)*128) and pun cols
            # [g*256, (g+1)*256).  Rank r of the pair gets rows [g*128 +
            # r*64, +64) and pun [g*256 + r*128, +128).
            last_ev = {}

            def emit_pj_group(g):
                for kb in range(g * KBG, (g + 1) * KBG):
                    ps_tiles = [
                        psum.tile([P, 512], F32, tag="pjps", name=f"pjps{kb}_{ch}")
                        for ch in range(CH)
                    ]
                    for s in range(NST):
                        lhsT = xsw8_tiles[s][:, :, kb * P : (kb + 1) * P]
                        for ch in range(CH):
                            nc.tensor.matmul(
                                ps_tiles[ch][:],
                                lhsT,
                                eu8_tiles[s][:, :, ch * 512 : (ch + 1) * 512],
                                start=(s == 0),
                                stop=(s == NST - 1),
                                perf_mode=DR,
                            )
                    r = kb - g * KBG
                    pj_view = rs_in[g][r * SUBCH : r * SUBCH + SUBPJ].rearrange(
                        "(k c) -> k c", c=C2
                    )
                    for ch in range(CH):
                        ev = pool_ev.tile([P, 512], BF16, tag="ev")
                        last_ev["copy"] = nc.vector.tensor_copy(ev[:], ps_tiles[ch][:])
                        last_ev["dma"] = nc.sync.dma_start(
                            pj_view[:, ch * 512 : (ch + 1) * 512], ev[:]
                        )
                # p_unseen partials for this group's 512-wide chunk
                pu_ps = psum_pu.tile([P, 512], F32, tag="pups", name=f"pups{g}")
                for s in range(NST):
                    nc.tensor.matmul(
                        pu_ps[:],
                        ones8[:],
                        eu8_tiles[s][:, :, g * 512 : (g + 1) * 512],
                        start=(s == 0),
                        stop=(s == NST - 1),
                        perf_mode=DR,
                    )
                ev = pool_evpu.tile([1, 512], BF16, tag="evpu")
                last_ev["copy"] = nc.vector.tensor_copy(ev[:], pu_ps[0:1, :])
                for r in range(2):
                    pun_view = rs_in[g][
                        r * SUBCH + SUBPJ : r * SUBCH + SUBPJ + PUNW
                    ].rearrange("(a c) -> a c", a=1)
                    last_ev["dma"] = nc.sync.dma_start(
                        pun_view[:], ev[:, r * PUNW : (r + 1) * PUNW]
                    )

                nc.gpsimd.collective_compute(
                    "ReduceScatter",
                    ALU.add,
                    replica_groups=[[0, 1], [2, 3], [4, 5], [6, 7]],
                    ins=[rs_in[g].opt()],
                    outs=[rs_out[g].opt()],
                )

            # -------------------- phase 3: entropies --------------------
            ones = acc.tile([P, 1], F32)
            nc.vector.memset(ones[:], 1.0)
            s1c = acc.tile([P, NG], F32)
            s2g = acc.tile([1, NG], F32)

            def emit_entropy_group(g):
                # order-only deps: keep the in-order DVE / SP queues free of
                # collective-dependent entropy work until every PSUM eviction
                # (which feeds the PE) has issued, else PE stalls behind the
                # collectives (head-of-line blocking).
                def after_ev(inst):
                    add_dep_helper(inst.ins, last_ev["copy"].ins, sync=False,
                                   reason="entropy after evictions")
                    return inst

                def after_ev_dma(inst):
                    add_dep_helper(inst.ins, last_ev["dma"].ins, sync=False,
                                   reason="entropy dma after eviction dmas")
                    return inst

                pj_t = pool_pjt.tile([P, C2], BF16, tag="pjt", name=f"pjt{g}")
                after_ev_dma(nc.sync.dma_start(
                    pj_t[:], rs_out[g][0:SUBPJ].rearrange("(p c) -> p c", c=C2)
                ))
                psn = stat.tile([P, 1], F32, tag="stat", name=f"psn{g}")
                after_ev(nc.vector.reduce_sum(psn[:], pj_t[:], axis=AX.X))
                psc = stat.tile([P, 1], F32, tag="stat", name=f"psc{g}")
                nc.vector.tensor_scalar_max(psc[:], psn[:], NEPS)
                lps = stat.tile([P, 1], F32, tag="stat", name=f"lps{g}")
                nc.scalar.activation(lps[:], psc[:], AF.Ln)

                rs_cl = stat.tile([P, 1], F32, tag="stat", name=f"rscl{g}")
                pcl = pool_pcl.tile([P, C2], F32)
                after_ev(nc.vector.tensor_scalar(
                    pcl[:], pj_t[:], NEPS, None, op0=ALU.max, op1=ALU.add,
                    accum_out=rs_cl[:],
                ))
                lp = pool_lp.tile([P, C2], F32)
                nc.scalar.activation(lp[:], pcl[:], AF.Ln)
                # NOTE: tensor_tensor_reduce wedges the exec unit on this
                # runtime (NRT_EXEC_UNIT_UNRECOVERABLE) — mult in place into
                # pcl, then reduce.
                a1 = stat.tile([P, 1], F32, tag="stat", name=f"a1_{g}")
                nc.vector.tensor_tensor(pcl[:], pcl[:], lp[:], op=ALU.mult)
                nc.vector.reduce_sum(a1[:], pcl[:], axis=AX.X)
                b1 = stat.tile([P, 1], F32, tag="stat", name=f"b1_{g}")
                nc.vector.tensor_tensor(b1[:], lps[:], rs_cl[:], op=ALU.mult)
                nc.vector.tensor_tensor(
                    s1c[:, g : g + 1], a1[:], b1[:], op=ALU.subtract
                )

                # p_unseen entropy for this group's final slice
                puf = pool_pu3.tile([1, PUNW], BF16, tag="puf", name=f"puf{g}")
                after_ev_dma(nc.sync.dma_start(
                    puf[:],
                    rs_out[g][SUBPJ : SUBPJ + PUNW].rearrange("(a c) -> a c", a=1),
                ))
                puc = pool_pu3.tile([1, PUNW], F32, tag="puc", name=f"puc{g}")
                after_ev(nc.vector.tensor_scalar_max(puc[:], puf[:], NEPS))
                lpu = pool_pu3.tile([1, PUNW], F32, tag="lpu", name=f"lpu{g}")
                nc.scalar.activation(lpu[:], puc[:], AF.Ln)
                # the N-scaling leaves an uncancelled -ln(N) on this term
                lpu2 = pool_pu3.tile([1, PUNW], F32, tag="lpu2", name=f"lpu2{g}")
                nc.vector.tensor_scalar(lpu2[:], lpu[:], -LN_N, None, op0=ALU.add)
                pup = pool_pu3.tile([1, PUNW], F32, tag="pup", name=f"pup{g}")
                nc.vector.tensor_tensor(pup[:], puc[:], lpu2[:], op=ALU.mult)
                nc.vector.reduce_sum(s2g[:, g : g + 1], pup[:], axis=AX.X)

            # all matmul groups first: the DVE stream must finish every PSUM
            # eviction before any collective-dependent entropy op, or the
            # in-order DVE queue stalls the PE behind the collectives.
            for g in range(NG):
                emit_pj_group(g)
            for g in range(NG):
                emit_entropy_group(g)

            # cross-partition total of s1, then emit [s1, s2]
            s1r = acc.tile([P, 1], F32)
            nc.vector.reduce_sum(s1r[:], s1c[:], axis=AX.X)
            ps_fin = psum_fin.tile([1, 1], F32, tag="fin")
            nc.tensor.matmul(ps_fin[:], s1r[:], ones[:])
            s2 = acc.tile([1, 1], F32)
            nc.vector.reduce_sum(s2[:], s2g[:], axis=AX.X)
            fin = acc.tile([1, 2], F32)
            nc.scalar.copy(fin[:, 0:1], ps_fin[:])
            nc.vector.tensor_copy(fin[:, 1:2], s2[:])
            nc.sync.dma_start(out_d[:], fin[:])

    nc.finalize()
    return nc


def _get_nc():
    global _NC
    if _NC is None:
        _NC = _build_nc()
    return _NC


def make_in_maps(x_seen, x_unseen):
    xs16 = x_seen.astype(np.float16)
    xu16 = x_unseen.astype(np.float16)
    in_maps = []
    for h in range(H):
        for v in range(V):
            in_maps.append(
                {
                    "xs": np.ascontiguousarray(xs16[v, h]),
                    "xu": np.ascontiguousarray(xu16[v, h]),
                }
            )
    return in_maps


def kernel(x_seen: np.ndarray, x_unseen: np.ndarray) -> np.ndarray:
    import os

    global LAST_RESULTS
    nc = _get_nc()
    in_maps = make_in_maps(x_seen, x_unseen)
    trace = os.environ.get("KERNEL_TRACE", "0") == "1"
    kw = {}
    td = os.environ.get("KERNEL_TRACE_DIR")
    if td:
        kw["tmpdir"] = td
    res = run_bass_kernel_spmd(nc, in_maps, list(range(H * V)), trace=trace, **kw)
    LAST_RESULTS = res
    s1 = sum(float(r["parts"][0, 0]) for r in res.results)
    s2 = sum(float(r["parts"][0, 1]) for r in res.results)
    return np.array((s2 - s1) / (H * N), dtype=np.float32)


# revision 26
# speedup vs baseline: 1.0433x; 1.0433x over previous
"""Trainium2 Bass kernel for nn_MI_35115652612725 (mutual-information loss).

Math (see reference): per h-slice,
  xs = softmax(x_seen[.,h]/T, -1)  (h, N, C1),  xu = softmax(x_unseen/T, -1)^T
  p_joint = xu @ xs / N;  p_seen/p_unseen are its column/row marginals.
  out = mean_h[ -sum p_joint*(log p_joint - log p_seen) + sum p_unseen*log p_unseen ]

Sharding: 8 cores = (h=4) x (v=2).  Core 2h+v processes the contiguous slabs
x_seen[v,h] (2048x1024) and x_unseen[v,h] (2048x2048); the N=v*B contraction
axis splits exactly along v.  Inputs are cast to fp16 on the host (the exp
argument 20*x loses <0.06 absolute, negligible through the softmax), halving
input HBM traffic.  Both matmul operands are row-normalized softmax numerators quantized to
fp8e4 (eu8 = eu/su, xsw8 = es/ss in [0,1]), run as DoubleRow fp8 matmuls
(256-deep contraction per instruction, 2x bf16 throughput), so the PSUM
partials equal N*p_joint.  p_unseen partials come from a ones-stationary
DoubleRow matmul (column sums of eu8).  Four pipelined ReduceScatters over core
pairs {2h,2h+1} sum the v-partials in bf16 and split the k axis for the
entropy phase.  The entropy phase works on the N-scaled values: clamps move
to N*EPS, the log N offsets cancel exactly in a1-b1, p_unseen's term
subtracts ln N explicitly, and the host divides the summed scalars by H*N.

A fixed -90 shift replaces the per-row max in softmax: any per-row constant
cancels exactly in all outputs here, and with |20*x| < 115 neither exp nor
the f32 sums can overflow (underflow only kills terms < e^-60 of the row
max, far below the fp8 quantization noise anyway).
"""

import math

import numpy as np

import concourse.bass as bass
import concourse.bacc as bacc
import concourse.mybir as mybir
from concourse import tile
from concourse.tile import add_dep_helper
from concourse.bass_utils import run_bass_kernel_spmd

F32 = mybir.dt.float32
F16 = mybir.dt.float16
BF16 = mybir.dt.bfloat16
FP8 = mybir.dt.float8e4
AF = mybir.ActivationFunctionType
ALU = mybir.AluOpType
AX = mybir.AxisListType
DR = mybir.MatmulPerfMode.DoubleRow

V, H, B, C1, C2 = 2, 4, 2048, 1024, 2048
N = V * B
P = 128
NST = B // (2 * P)     # 8 super-tiles of the contraction axis (256 rows each)
KB = C1 // P           # 8 stationary blocks (k on psum partitions)
CH = C2 // 512         # 4 moving chunks (c on psum free axis)
SCALE = 20.0           # 1/TEMP
SHIFT = -90.0
EPS = 1e-7
NEPS = float(EPS * N)  # clamp for the N-scaled p_joint / p_unseen values
LN_N = float(math.log(N))

NG = 4                 # pipelined collective groups (2 kb blocks each)
KBG = KB // NG         # kb blocks per group (2)
SUBPJ = P * C2         # per-rank pjT piece: 128*2048 elems
PUNW = C2 // NG // 2   # per-rank p_unseen piece: 256
SUBCH = SUBPJ + PUNW

_NC = None
LAST_RESULTS = None


def _build_nc():
    nc = bacc.Bacc(None, num_devices=8)
    # Register the exp shift as a preamble const AP (memset + barrier before
    # any tile instruction) so the Exp activations don't pick up an extra
    # sync-wait on a bias-producing instruction — the ACT instruction
    # encoding only has room for one wait here.
    shift_t = nc.alloc_sbuf_tensor(f"const-float32-{SHIFT}", [128, 1], F32)
    nc.gpsimd.memset(shift_t.ap(), SHIFT)
    nc.const_aps.aps[(F32, SHIFT)] = shift_t.ap()
    nc.all_engine_barrier()

    xs_d = nc.dram_tensor("xs", [B, C1], F16, kind="ExternalInput")
    xu_d = nc.dram_tensor("xu", [B, C2], F16, kind="ExternalInput")
    out_d = nc.dram_tensor("parts", [1, 2], F32, kind="ExternalOutput")

    with tile.TileContext(nc) as tc:
        with (
            tc.tile_pool(name="dram", bufs=1, space="DRAM") as dram,
            tc.tile_pool(name="xu_raw", bufs=2) as pool_xu,
            tc.tile_pool(name="xs_raw", bufs=2) as pool_xs,
            tc.tile_pool(name="eb", bufs=2) as pool_eb,
            tc.tile_pool(name="sb", bufs=2) as pool_sb,
            tc.tile_pool(name="eu8", bufs=NST) as pool_eu8,
            tc.tile_pool(name="xsw8", bufs=NST) as pool_xsw8,
            tc.tile_pool(name="stat", bufs=8 * NST) as stat,
            tc.tile_pool(name="psum", bufs=6, space="PSUM") as psum,
            tc.tile_pool(name="psum_fin", bufs=1, space="PSUM") as psum_fin,
            tc.tile_pool(name="psum_pu", bufs=1, space="PSUM") as psum_pu,
            tc.tile_pool(name="pjt", bufs=2) as pool_pjt,
            tc.tile_pool(name="pcl", bufs=2) as pool_pcl,
            tc.tile_pool(name="lp", bufs=1) as pool_lp,
            tc.tile_pool(name="evict", bufs=4) as pool_ev,
            tc.tile_pool(name="evict_pu", bufs=2) as pool_evpu,
            tc.tile_pool(name="pu3", bufs=2) as pool_pu3,
            tc.tile_pool(name="acc", bufs=1) as acc,
        ):
            rs_in = [
                dram.tile([2 * SUBCH], BF16, name=f"rs_in{g}") for g in range(NG)
            ]
            rs_out = [dram.tile([SUBCH], BF16, name=f"rs_out{g}") for g in range(NG)]

            # super-tile s, sub-row i, partition p covers input row
            # n = 256*s + 128*i + p — the DoubleRow contraction pairing.
            xu_t = xu_d[:].rearrange("(s i p) c -> s p i c", i=2, p=P)
            xs_t = xs_d[:].rearrange("(s i p) c -> s p i c", i=2, p=P)

            # fp8 ones stationary for the p_unseen column-sum matmul.  The
            # dual-fp8 ldweights ISA check rejects narrow stationaries, so
            # use the same [128, 2, 128] shape as the pjT stationaries — all
            # 128 output rows hold the identical column sum; evict row 0.
            ones8 = acc.tile([P, 2, P], FP8)
            nc.vector.memset(ones8[:], 1.0)

            # ---------------- phase 1: exp + normalize to fp8 ----------------
            eu8_tiles, xsw8_tiles = [], []
            for s in range(NST):
                xu_raw = pool_xu.tile([P, 2, C2], F16)
                nc.sync.dma_start(xu_raw[:], xu_t[s])
                xs_raw = pool_xs.tile([P, 2, C1], F16)
                nc.sync.dma_start(xs_raw[:], xs_t[s])

                eu8 = pool_eu8.tile([P, 2, C2], FP8, tag="eu8", name=f"eu8_{s}")
                xsw8 = pool_xsw8.tile([P, 2, C1], FP8, tag="xsw8", name=f"xsw8_{s}")
                for i in range(2):
                    eb = pool_eb.tile([P, C2], BF16)
                    su = stat.tile([P, 1], F32, tag="stat", name=f"su{s}_{i}")
                    nc.scalar.activation(
                        eb[:], xu_raw[:, i, :], AF.Exp,
                        bias=SHIFT, scale=SCALE, accum_out=su[:],
                    )
                    wu = stat.tile([P, 1], F32, tag="stat", name=f"wu{s}_{i}")
                    nc.vector.reciprocal(wu[:], su[:])
                    nc.vector.tensor_scalar_mul(eu8[:, i, :], eb[:], wu[:])

                    sb = pool_sb.tile([P, C1], BF16)
                    ss = stat.tile([P, 1], F32, tag="stat", name=f"ss{s}_{i}")
                    nc.scalar.activation(
                        sb[:], xs_raw[:, i, :], AF.Exp,
                        bias=SHIFT, scale=SCALE, accum_out=ss[:],
                    )
                    ws = stat.tile([P, 1], F32, tag="stat", name=f"ws{s}_{i}")
                    nc.vector.reciprocal(ws[:], ss[:])
                    nc.vector.tensor_scalar_mul(xsw8[:, i, :], sb[:], ws[:])

                eu8_tiles.append(eu8)
                xsw8_tiles.append(xsw8)

            # ---------------- phase 2 + overlapped collectives ----------------
            # group g = kb block g: pjT rows [g*128, (g+1)*128) and pun cols
            # [g*256, (g+1)*256).  Rank r of the pair gets rows [g*128 +
            # r*64, +64) and pun [g*256 + r*128, +128).
            last_ev = {}

            def emit_pj_group(g):
                for kb in range(g * KBG, (g + 1) * KBG):
                    ps_tiles = [
                        psum.tile([P, 512], F32, tag="pjps", name=f"pjps{kb}_{ch}")
                        for ch in range(CH)
                    ]
                    for s in range(NST):
                        lhsT = xsw8_tiles[s][:, :, kb * P : (kb + 1) * P]
                        for ch in range(CH):
                            nc.tensor.matmul(
                                ps_tiles[ch][:],
                                lhsT,
                                eu8_tiles[s][:, :, ch * 512 : (ch + 1) * 512],
                                start=(s == 0),
                                stop=(s == NST - 1),
                                perf_mode=DR,
                            )
                    r = kb - g * KBG
                    pj_view = rs_in[g][r * SUBCH : r * SUBCH + SUBPJ].rearrange(
                        "(k c) -> k c", c=C2
                    )
                    for ch in range(CH):
                        ev = pool_ev.tile([P, 512], BF16, tag="ev")
                        last_ev["copy"] = nc.vector.tensor_copy(ev[:], ps_tiles[ch][:])
                        last_ev["dma"] = nc.sync.dma_start(
                            pj_view[:, ch * 512 : (ch + 1) * 512], ev[:]
                        )
                # p_unseen partials for this group's 512-wide chunk
                pu_ps = psum_pu.tile([P, 512], F32, tag="pups", name=f"pups{g}")
                for s in range(NST):
                    nc.tensor.matmul(
                        pu_ps[:],
                        ones8[:],
                        eu8_tiles[s][:, :, g * 512 : (g + 1) * 512],
                        start=(s == 0),
                        stop=(s == NST - 1),
                        perf_mode=DR,
                    )
                ev = pool_evpu.tile([1, 512], BF16, tag="evpu")
                last_ev["copy"] = nc.vector.tensor_copy(ev[:], pu_ps[0:1, :])
                for r in range(2):
                    pun_view = rs_in[g][
                        r * SUBCH + SUBPJ : r * SUBCH + SUBPJ + PUNW
                    ].rearrange("(a c) -> a c", a=1)
                    last_ev["dma"] = nc.sync.dma_start(
                        pun_view[:], ev[:, r * PUNW : (r + 1) * PUNW]
                    )

                nc.gpsimd.collective_compute(
                    "ReduceScatter",
                    ALU.add,
                    replica_groups=[[0, 1], [2, 3], [4, 5], [6, 7]],
                    ins=[rs_in[g].opt()],
                    outs=[rs_out[g].opt()],
                )

            # -------------------- phase 3: entropies --------------------
            ones = acc.tile([P, 1], F32)
            nc.vector.memset(ones[:], 1.0)
            s1c = acc.tile([P, NG], F32)
            s2g = acc.tile([1, NG], F32)

            def emit_entropy_group(g):
                # order-only deps: keep the in-order DVE / SP queues free of
                # collective-dependent entropy work until every PSUM eviction
                # (which feeds the PE) has issued, else PE stalls behind the
                # collectives (head-of-line blocking).
                def after_ev(inst):
                    add_dep_helper(inst.ins, last_ev["copy"].ins, sync=False,
                                   reason="entropy after evictions")
                    return inst

                def after_ev_dma(inst):
                    add_dep_helper(inst.ins, last_ev["dma"].ins, sync=False,
                                   reason="entropy dma after eviction dmas")
                    return inst

                pj_t = pool_pjt.tile([P, C2], BF16, tag="pjt", name=f"pjt{g}")
                after_ev_dma(nc.sync.dma_start(
                    pj_t[:], rs_out[g][0:SUBPJ].rearrange("(p c) -> p c", c=C2)
                ))
                psn = stat.tile([P, 1], F32, tag="stat", name=f"psn{g}")
                after_ev(nc.vector.reduce_sum(psn[:], pj_t[:], axis=AX.X))
                psc = stat.tile([P, 1], F32, tag="stat", name=f"psc{g}")
                nc.vector.tensor_scalar_max(psc[:], psn[:], NEPS)
                lps = stat.tile([P, 1], F32, tag="stat", name=f"lps{g}")
                nc.scalar.activation(lps[:], psc[:], AF.Ln)

                rs_cl = stat.tile([P, 1], F32, tag="stat", name=f"rscl{g}")
                pcl = pool_pcl.tile([P, C2], F32)
                after_ev(nc.vector.tensor_scalar(
                    pcl[:], pj_t[:], NEPS, None, op0=ALU.max, op1=ALU.add,
                    accum_out=rs_cl[:],
                ))
                lp = pool_lp.tile([P, C2], F32)
                nc.scalar.activation(lp[:], pcl[:], AF.Ln)
                # NOTE: tensor_tensor_reduce wedges the exec unit on this
                # runtime (NRT_EXEC_UNIT_UNRECOVERABLE) — mult in place into
                # pcl, then reduce.
                a1 = stat.tile([P, 1], F32, tag="stat", name=f"a1_{g}")
                nc.vector.tensor_tensor(pcl[:], pcl[:], lp[:], op=ALU.mult)
                nc.vector.reduce_sum(a1[:], pcl[:], axis=AX.X)
                b1 = stat.tile([P, 1], F32, tag="stat", name=f"b1_{g}")
                nc.vector.tensor_tensor(b1[:], lps[:], rs_cl[:], op=ALU.mult)
                nc.vector.tensor_tensor(
                    s1c[:, g : g + 1], a1[:], b1[:], op=ALU.subtract
                )

                # p_unseen entropy for this group's final slice
                puf = pool_pu3.tile([1, PUNW], BF16, tag="puf", name=f"puf{g}")
                after_ev_dma(nc.sync.dma_start(
                    puf[:],
                    rs_out[g][SUBPJ : SUBPJ + PUNW].rearrange("(a c) -> a c", a=1),
                ))
                puc = pool_pu3.tile([1, PUNW], F32, tag="puc", name=f"puc{g}")
                after_ev(nc.vector.tensor_scalar_max(puc[:], puf[:], NEPS))
                lpu = pool_pu3.tile([1, PUNW], F32, tag="lpu", name=f"lpu{g}")
                nc.scalar.activation(lpu[:], puc[:], AF.Ln)
                # the N-scaling leaves an uncancelled -ln(N) on this term
                lpu2 = pool_pu3.tile([1, PUNW], F32, tag="lpu2", name=f"lpu2{g}")
                nc.vector.tensor_scalar(lpu2[:], lpu[:], -LN_N, None, op0=ALU.add)
                pup = pool_pu3.tile([1, PUNW], F32, tag="pup", name=f"pup{g}")
                nc.vector.tensor_tensor(pup[:], puc[:], lpu2[:], op=ALU.mult)
                nc.vector.reduce_sum(s2g[:, g : g + 1], pup[:], axis=AX.X)

            # all matmul groups first: the DVE stream must finish every PSUM
            # eviction before any collective-dependent entropy op, or the
            # in-order DVE queue stalls the PE behind the collectives.
            for g in range(NG):
                emit_pj_group(g)
            for g in range(NG):
                emit_entropy_group(g)

            # cross-partition total of s1, then emit [s1, s2]
            s1r = acc.tile([P, 1], F32)
            nc.vector.reduce_sum(s1r[:], s1c[:], axis=AX.X)
            ps_fin = psum_fin.tile([1, 1], F32, tag="fin")
            nc.tensor.matmul(ps_fin[:], s1r[:], ones[:])
            s2 = acc.tile([1, 1], F32)
            nc.vector.reduce_sum(s2[:], s2g[:], axis=AX.X)
            fin = acc.tile([1, 2], F32)
            nc.scalar.copy(fin[:, 0:1], ps_fin[:])
            nc.vector.tensor_copy(fin[:, 1:2], s2[:])
            nc.sync.dma_start(out_d[:], fin[:])

    nc.finalize()
    return nc


def _get_nc():
    global _NC
    if _NC is None:
        _NC = _build_nc()
    return _NC


def make_in_maps(x_seen, x_unseen):
    xs16 = x_seen.astype(np.float16)
    xu16 = x_unseen.astype(np.float16)
    in_maps = []
    for h in range(H):
        for v in range(V):
            in_maps.append(
                {
                    "xs": np.ascontiguousarray(xs16[v, h]),
                    "xu": np.ascontiguousarray(xu16[v, h]),
                }
            )
    return in_maps


def kernel(x_seen: np.ndarray, x_unseen: np.ndarray) -> np.ndarray:
    import os

    global LAST_RESULTS
    nc = _get_nc()
    in_maps = make_in_maps(x_seen, x_unseen)
    trace = os.environ.get("KERNEL_TRACE", "0") == "1"
    kw = {}
    td = os.environ.get("KERNEL_TRACE_DIR")
    if td:
        kw["tmpdir"] = td
    res = run_bass_kernel_spmd(nc, in_maps, list(range(H * V)), trace=trace, **kw)
    LAST_RESULTS = res
    s1 = sum(float(r["parts"][0, 0]) for r in res.results)
    s2 = sum(float(r["parts"][0, 1]) for r in res.results)
    return np.array((s2 - s1) / (H * N), dtype=np.float32)
